# revision 2
# baseline (speedup 1.0000x reference)
"""MoE block (top-1 routing, shared FFN + per-expert LoRA) on 8 TRN2 NeuronCores.

Strategy: data-parallel over the 8192 tokens (1024 tokens/core), weights
replicated. The reference's dense-then-mask expert loop collapses to:

    logits = x @ gate_W.T + gate_b ; e* = argmax(logits)        (fp32)
    u      = x @ A_cat.T                 [N, 32]                (bf16)
    u_m    = u * onehot-mask(e*)  (zero all but selected expert's 4 lora rows)
    inter  = relu(x @ wi_W.T + u_m @ B_cat + wi_b)              (bf16 matmul)
    out    = inter @ wo_W.T + wo_b                              (bf16 matmul)

Everything runs in transposed (feature-major) layout on chip; the host
pre-transposes the shards/weights and re-transposes the output.
"""

import numpy as np
import ml_dtypes
from contextlib import ExitStack

import concourse.bass as bass
import concourse.tile as tile
from concourse import bacc, mybir
from concourse.bass_utils import run_bass_kernel_spmd
from concourse.masks import make_identity

F32 = mybir.dt.float32
BF16 = mybir.dt.bfloat16
U32 = mybir.dt.uint32
BF = ml_dtypes.bfloat16

B, S, D, F, E, R = 4, 2048, 1024, 4096, 8, 4
NCORES = 8
NT = B * S          # 8192 tokens total
N = NT // NCORES    # 1024 tokens per core
ER = E * R          # 32 lora rows
KD = D // 128       # 8 contraction tiles over D
KF = F // 128       # 32 contraction tiles over F
TT = N // 128       # 8 token tiles (routing)
TH = N // 512       # 2 token halves (matmul moving dim)
P = 128

Relu = mybir.ActivationFunctionType.Relu


def _emit(ctx: ExitStack, tc: tile.TileContext, io: dict):
    nc = tc.nc

    consts = ctx.enter_context(tc.tile_pool(name="consts", bufs=1))
    xpool = ctx.enter_context(tc.tile_pool(name="xpool", bufs=1))
    wipool = ctx.enter_context(tc.tile_pool(name="wipool", bufs=1))
    ipool = ctx.enter_context(tc.tile_pool(name="ipool", bufs=1))
    x32p = ctx.enter_context(tc.tile_pool(name="x32p", bufs=4))
    wop = ctx.enter_context(tc.tile_pool(name="wop", bufs=2))
    rwork = ctx.enter_context(tc.tile_pool(name="rwork", bufs=2))
    outp = ctx.enter_context(tc.tile_pool(name="outp", bufs=3))
    sps = ctx.enter_context(tc.tile_pool(name="sps", bufs=1, space="PSUM"))
    cps = ctx.enter_context(tc.tile_pool(name="cps", bufs=3, space="PSUM"))
    dps = ctx.enter_context(tc.tile_pool(name="dps", bufs=2, space="PSUM"))

    # ---------- constants ----------
    identity = consts.tile([P, P], F32, tag="identity")
    make_identity(nc, identity)
    # econst[p, e*R + r] = e  (expert id per lora row, replicated on free axis)
    econst = consts.tile([P, ER], F32, tag="econst")
    for e in range(E):
        nc.vector.memset(econst[:, e * R:(e + 1) * R], float(e))
    gateb_sb = consts.tile([P, E], F32, tag="gb")
    nc.sync.dma_start(out=gateb_sb, in_=io["gb"].partition_broadcast(P))
    wib_sb = consts.tile([P, KF], F32, tag="wib")  # col f = wi_b[f*128 + p]
    nc.sync.dma_start(out=wib_sb, in_=io["wib"].rearrange("(a p) -> p a", p=P))
    wob_sb = consts.tile([P, KD], F32, tag="wob")
    nc.sync.dma_start(out=wob_sb, in_=io["wob"].rearrange("(a p) -> p a", p=P))

    gate_sb = []
    acat_sb = []
    for k in range(KD):
        g = consts.tile([P, E], F32, tag=f"gt{k}")
        nc.sync.dma_start(out=g, in_=io["gT"][k * P:(k + 1) * P, :])
        gate_sb.append(g)
        a = consts.tile([P, ER], BF16, tag=f"ac{k}")
        nc.sync.dma_start(out=a, in_=io["aT"][k * P:(k + 1) * P, :])
        acat_sb.append(a)
    bcat_sb = consts.tile([ER, F], BF16, tag="bc")
    nc.sync.dma_start(out=bcat_sb, in_=io["bT"])

    # ---------- resident activations / weights ----------
    x16 = []
    for k in range(KD):
        t = xpool.tile([P, N], BF16, tag=f"x16_{k}")
        nc.sync.dma_start(out=t, in_=io["xT16"][k * P:(k + 1) * P, :])
        x16.append(t)
    wi_sb = []
    for k in range(KD):
        t = wipool.tile([P, F], BF16, tag=f"wi{k}")
        nc.sync.dma_start(out=t, in_=io["wiT"][k * P:(k + 1) * P, :])
        wi_sb.append(t)
    inter_sb = [ipool.tile([P, N], BF16, tag=f"inter{f}", name=f"inter{f}")
                for f in range(KF)]

    maskT4 = consts.tile([ER, N], F32, tag="maskT4")
    um16 = [consts.tile([ER, 512], BF16, tag=f"um{th}", name=f"um{th}")
            for th in range(TH)]

    # ---------- router (fp32) ----------
    for tt in range(TT):
        psum_l = sps.tile([P, E], F32, tag="pl")
        for k in range(KD):
            x32 = x32p.tile([P, P], F32, tag="x32")
            nc.sync.dma_start(
                out=x32, in_=io["xT32"][k * P:(k + 1) * P, tt * P:(tt + 1) * P])
            nc.tensor.matmul(psum_l, lhsT=x32, rhs=gate_sb[k],
                             start=(k == 0), stop=(k == KD - 1))
        logits = rwork.tile([P, E], F32, tag="lg")
        nc.vector.tensor_add(logits, psum_l, gateb_sb)
        max8 = rwork.tile([P, E], F32, tag="mx")
        nc.vector.max(out=max8, in_=logits)
        idx8 = rwork.tile([P, E], U32, tag="ix")
        nc.vector.max_index(idx8, max8, logits)
        idxf = rwork.tile([P, 1], F32, tag="if")
        nc.vector.tensor_copy(idxf, idx8[:, 0:1])
        # mask_rep[t, e*R+r] = (argmax == e)
        mask_rep = rwork.tile([P, ER], F32, tag="mr")
        nc.vector.tensor_scalar(mask_rep, econst, idxf, None,
                                mybir.AluOpType.is_equal)
        psum_m = sps.tile([ER, P], F32, tag="pm")
        nc.tensor.transpose(psum_m, mask_rep, identity)
        nc.vector.tensor_copy(maskT4[:, tt * P:(tt + 1) * P], psum_m)

    # ---------- masked lora projection u (bf16) ----------
    for th in range(TH):
        ts = slice(th * 512, (th + 1) * 512)
        psum_u = sps.tile([ER, 512], F32, tag="pu")
        for k in range(KD):
            nc.tensor.matmul(psum_u, lhsT=acat_sb[k], rhs=x16[k][:, ts],
                             start=(k == 0), stop=(k == KD - 1))
        nc.vector.tensor_mul(um16[th], psum_u, maskT4[:, ts])

    # ---------- matmul 1: interT = relu(wi @ x.T + Bcat.T @ u_m + wi_b) ----------
    for th in range(TH):
        ts = slice(th * 512, (th + 1) * 512)
        for f in range(KF):
            psum1 = cps.tile([P, 512], F32, tag="p1")
            for k in range(KD):
                nc.tensor.matmul(psum1, lhsT=wi_sb[k][:, f * P:(f + 1) * P],
                                 rhs=x16[k][:, ts], start=(k == 0), stop=False)
            nc.tensor.matmul(psum1, lhsT=bcat_sb[:, f * P:(f + 1) * P],
                             rhs=um16[th], start=False, stop=True)
            nc.scalar.activation(inter_sb[f][:, ts], psum1, Relu,
                                 bias=wib_sb[:, f:f + 1])

    # ---------- matmul 2: outT = wo @ inter + wo_b ----------
    for d in range(KD):
        wo_tiles = []
        for kf in range(KF):
            w = wop.tile([P, P], BF16, tag=f"wo{kf}")
            nc.sync.dma_start(
                out=w, in_=io["woT"][kf * P:(kf + 1) * P, d * P:(d + 1) * P])
            wo_tiles.append(w)
        for th in range(TH):
            ts = slice(th * 512, (th + 1) * 512)
            psum2 = dps.tile([P, 512], F32, tag="p2")
            for kf in range(KF):
                nc.tensor.matmul(psum2, lhsT=wo_tiles[kf], rhs=inter_sb[kf][:, ts],
                                 start=(kf == 0), stop=(kf == KF - 1))
            osb = outp.tile([P, 512], F32, tag="osb")
            nc.vector.tensor_scalar(osb, psum2, wob_sb[:, d:d + 1], None,
                                    mybir.AluOpType.add)
            nc.sync.dma_start(out=io["outT"][d * P:(d + 1) * P, ts], in_=osb)


_CACHED_NC = None


def build_nc():
    global _CACHED_NC
    if _CACHED_NC is not None:
        return _CACHED_NC
    nc = bacc.Bacc("TRN2", target_bir_lowering=False, debug=False,
                   enable_asserts=False, num_devices=NCORES)
    decls = [
        ("xT32", [D, N], F32, False),
        ("xT16", [D, N], BF16, False),
        ("gT", [D, E], F32, False),
        ("gb", [E], F32, False),
        ("aT", [D, ER], BF16, False),
        ("bT", [ER, F], BF16, False),
        ("wiT", [D, F], BF16, False),
        ("wib", [F], F32, False),
        ("woT", [F, D], BF16, False),
        ("wob", [D], F32, False),
        ("outT", [D, N], F32, True),
    ]
    io = {}
    for name, shape, dt_, is_out in decls:
        io[name] = nc.dram_tensor(
            name, shape, dt_, kind="ExternalOutput" if is_out else "ExternalInput"
        ).ap()
    with tile.TileContext(nc) as tc:
        with ExitStack() as ctx:
            _emit(ctx, tc, io)
    nc.compile()
    _CACHED_NC = nc
    return nc


def make_in_maps(inputs: dict) -> list[dict]:
    f32 = np.float32
    x = np.ascontiguousarray(np.asarray(inputs["hidden_states"], f32).reshape(NT, D))
    gT = np.ascontiguousarray(np.asarray(inputs["gate_W"], f32).T)          # [D, E]
    gb = np.ascontiguousarray(np.asarray(inputs["gate_b"], f32))            # [E]
    aT = np.ascontiguousarray(
        np.asarray(inputs["lora_A"], f32).reshape(ER, D).T.astype(BF))      # [D, 32]
    bT = np.ascontiguousarray(
        np.asarray(inputs["lora_B"], f32).transpose(0, 2, 1).reshape(ER, F).astype(BF))
    wiT = np.ascontiguousarray(np.asarray(inputs["wi_W"], f32).T.astype(BF))  # [D, F]
    wib = np.ascontiguousarray(np.asarray(inputs["wi_b"], f32))             # [F]
    woT = np.ascontiguousarray(np.asarray(inputs["wo_W"], f32).T.astype(BF))  # [F, D]
    wob = np.ascontiguousarray(np.asarray(inputs["wo_b"], f32))             # [D]

    in_maps = []
    for c in range(NCORES):
        xT32 = np.ascontiguousarray(x[c * N:(c + 1) * N].T)                 # [D, N]
        in_maps.append({
            "xT32": xT32,
            "xT16": np.ascontiguousarray(xT32.astype(BF)),
            "gT": gT, "gb": gb, "aT": aT, "bT": bT,
            "wiT": wiT, "wib": wib, "woT": woT, "wob": wob,
        })
    return in_maps


def kernel(**inputs) -> np.ndarray:
    nc = build_nc()
    in_maps = make_in_maps(inputs)
    res = run_bass_kernel_spmd(nc, in_maps, core_ids=list(range(NCORES)))
    out = np.empty((NT, D), np.float32)
    for c in range(NCORES):
        out[c * N:(c + 1) * N] = res.results[c]["outT"].T
    return out.reshape(B, S, D)


# revision 4
# speedup vs baseline: 1.0620x; 1.0620x over previous
"""MoE block (top-1 routing, shared FFN + per-expert LoRA) on 8 TRN2 NeuronCores.

Strategy: data-parallel over the 8192 tokens (1024 tokens/core), weights
replicated. The reference's dense-then-mask expert loop collapses to:

    logits = x @ gate_W.T + gate_b ; e* = argmax(logits)        (fp32)
    u      = x @ A_cat.T                 [N, 32]                (bf16)
    u_m    = u * onehot-mask(e*)  (zero all but selected expert's 4 lora rows)
    inter  = relu(x @ wi_W.T + u_m @ B_cat + wi_b)              (bf16 matmul)
    out    = inter @ wo_W.T + wo_b                              (bf16 matmul)

Everything runs in transposed (feature-major) layout on chip; the host
pre-transposes the shards/weights and re-transposes the output.
"""

import numpy as np
import ml_dtypes
from contextlib import ExitStack

import concourse.bass as bass
import concourse.tile as tile
from concourse import bacc, mybir
from concourse.bass_utils import run_bass_kernel_spmd
from concourse.masks import make_identity

F32 = mybir.dt.float32
BF16 = mybir.dt.bfloat16
U32 = mybir.dt.uint32
BF = ml_dtypes.bfloat16

B, S, D, F, E, R = 4, 2048, 1024, 4096, 8, 4
NCORES = 8
NT = B * S          # 8192 tokens total
N = NT // NCORES    # 1024 tokens per core
ER = E * R          # 32 lora rows
KD = D // 128       # 8 contraction tiles over D
KF = F // 128       # 32 contraction tiles over F
TT = N // 128       # 8 token tiles (routing)
TH = N // 512       # 2 token halves (matmul moving dim)
P = 128

Relu = mybir.ActivationFunctionType.Relu


def _emit(ctx: ExitStack, tc: tile.TileContext, io: dict):
    nc = tc.nc

    consts = ctx.enter_context(tc.tile_pool(name="consts", bufs=1))
    xpool = ctx.enter_context(tc.tile_pool(name="xpool", bufs=1))
    wipool = ctx.enter_context(tc.tile_pool(name="wipool", bufs=1))
    ipool = ctx.enter_context(tc.tile_pool(name="ipool", bufs=1))
    x32p = ctx.enter_context(tc.tile_pool(name="x32p", bufs=4))
    wop = ctx.enter_context(tc.tile_pool(name="wop", bufs=2))
    rwork = ctx.enter_context(tc.tile_pool(name="rwork", bufs=2))
    outp = ctx.enter_context(tc.tile_pool(name="outp", bufs=3))
    sps = ctx.enter_context(tc.tile_pool(name="sps", bufs=1, space="PSUM"))
    cps = ctx.enter_context(tc.tile_pool(name="cps", bufs=3, space="PSUM"))
    dps = ctx.enter_context(tc.tile_pool(name="dps", bufs=2, space="PSUM"))

    # ---------- constants ----------
    identity = consts.tile([P, P], F32, tag="identity")
    make_identity(nc, identity)
    # econst[p, e*R + r] = e  (expert id per lora row, replicated on free axis)
    econst = consts.tile([P, ER], F32, tag="econst")
    for e in range(E):
        nc.vector.memset(econst[:, e * R:(e + 1) * R], float(e))
    gateb_sb = consts.tile([P, E], F32, tag="gb")
    nc.sync.dma_start(out=gateb_sb, in_=io["gb"].partition_broadcast(P))
    wib_sb = consts.tile([P, KF], F32, tag="wib")  # col f = wi_b[f*128 + p]
    nc.sync.dma_start(out=wib_sb, in_=io["wib"].rearrange("(a p) -> p a", p=P))
    wob_sb = consts.tile([P, KD], F32, tag="wob")
    nc.sync.dma_start(out=wob_sb, in_=io["wob"].rearrange("(a p) -> p a", p=P))

    gate_sb = []
    acat_sb = []
    for k in range(KD):
        g = consts.tile([P, E], F32, tag=f"gt{k}")
        nc.sync.dma_start(out=g, in_=io["gT"][k * P:(k + 1) * P, :])
        gate_sb.append(g)
        a = consts.tile([P, ER], BF16, tag=f"ac{k}")
        nc.sync.dma_start(out=a, in_=io["aT"][k * P:(k + 1) * P, :])
        acat_sb.append(a)
    bcat_sb = consts.tile([ER, F], BF16, tag="bc")
    nc.sync.dma_start(out=bcat_sb, in_=io["bT"])

    # ---------- PE warm-up: release the HAM clock gate while DMAs land ----------
    warm_src = consts.tile([P, 512], BF16, tag="warm")
    nc.vector.memset(warm_src, 1.0)
    for w in range(12):
        psum_w = cps.tile([P, 512], F32, tag="p1", name=f"pw{w}")
        nc.tensor.matmul(psum_w, lhsT=warm_src[:, 0:P], rhs=warm_src,
                         start=True, stop=True)

    # ---------- resident activations / weights ----------
    inter_sb = [ipool.tile([P, N], BF16, tag=f"inter{f}", name=f"inter{f}")
                for f in range(KF)]
    maskT4 = consts.tile([ER, N], F32, tag="maskT4")
    um16 = [consts.tile([ER, 512], BF16, tag=f"um{th}", name=f"um{th}")
            for th in range(TH)]

    # ---------- router (fp32, feature-major then transpose) ----------
    for th in range(TH):
        ts = slice(th * 512, (th + 1) * 512)
        psum_lt = sps.tile([E, 512], F32, tag="plt")
        x32t = []
        for k in range(KD):
            x32 = x32p.tile([P, 512], F32, tag="x32", name=f"x32_{th}_{k}")
            nc.sync.dma_start(out=x32, in_=io["xT32"][k * P:(k + 1) * P, ts])
            x32t.append(x32)
        for k in range(KD):
            nc.tensor.matmul(psum_lt, lhsT=gate_sb[k], rhs=x32t[k],
                             start=(k == 0), stop=(k == KD - 1))
        logitsT = rwork.tile([E, 512], F32, tag="lgT")
        nc.vector.tensor_copy(logitsT, psum_lt)
        for q in range(4):
            tt = th * 4 + q
            # transpose [8, 128] logit chunk to token-major [128, 8]
            psum_tr = sps.tile([P, E], F32, tag="pmask", name=f"ptr{tt}")
            nc.tensor.matmul(psum_tr, lhsT=logitsT[:, q * P:(q + 1) * P],
                             rhs=identity[0:E, 0:E], is_transpose=True,
                             start=True, stop=True)
            logits = rwork.tile([P, E], F32, tag="lg")
            nc.vector.tensor_add(logits, psum_tr, gateb_sb)
            max8 = rwork.tile([P, E], F32, tag="mx")
            nc.vector.max(out=max8, in_=logits)
            idx8 = rwork.tile([P, E], U32, tag="ix")
            nc.vector.max_index(idx8, max8, logits)
            idxf = rwork.tile([P, 1], F32, tag="if")
            nc.vector.tensor_copy(idxf, idx8[:, 0:1])
            # mask_rep[t, e*R+r] = (argmax == e)
            mask_rep = rwork.tile([P, ER], F32, tag="mr")
            nc.vector.tensor_scalar(mask_rep, econst, idxf, None,
                                    mybir.AluOpType.is_equal)
            psum_m = sps.tile([ER, P], F32, tag="pmask", name=f"pm{tt}")
            nc.tensor.transpose(psum_m, mask_rep, identity)
            nc.vector.tensor_copy(maskT4[:, tt * P:(tt + 1) * P], psum_m)

    # bulk loads, emitted after the router stream so its DMAs queue first
    x16 = []
    for k in range(KD):
        t = xpool.tile([P, N], BF16, tag=f"x16_{k}")
        nc.sync.dma_start(out=t, in_=io["xT16"][k * P:(k + 1) * P, :])
        x16.append(t)
    wi_sb = []
    for k in range(KD):
        t = wipool.tile([P, F], BF16, tag=f"wi{k}")
        nc.sync.dma_start(out=t, in_=io["wiT"][k * P:(k + 1) * P, :])
        wi_sb.append(t)

    # ---------- masked lora projection u (bf16) ----------
    for th in range(TH):
        ts = slice(th * 512, (th + 1) * 512)
        psum_u = sps.tile([ER, 512], F32, tag="pu")
        for k in range(KD):
            nc.tensor.matmul(psum_u, lhsT=acat_sb[k], rhs=x16[k][:, ts],
                             start=(k == 0), stop=(k == KD - 1))
        nc.vector.tensor_mul(um16[th], psum_u, maskT4[:, ts])

    # ---------- matmul 1: interT = relu(wi @ x.T + Bcat.T @ u_m + wi_b) ----------
    for th in range(TH):
        ts = slice(th * 512, (th + 1) * 512)
        for f in range(KF):
            psum1 = cps.tile([P, 512], F32, tag="p1")
            for k in range(KD):
                nc.tensor.matmul(psum1, lhsT=wi_sb[k][:, f * P:(f + 1) * P],
                                 rhs=x16[k][:, ts], start=(k == 0), stop=False)
            nc.tensor.matmul(psum1, lhsT=bcat_sb[:, f * P:(f + 1) * P],
                             rhs=um16[th], start=False, stop=True)
            nc.scalar.activation(inter_sb[f][:, ts], psum1, Relu,
                                 bias=wib_sb[:, f:f + 1])

    # ---------- matmul 2: outT = wo @ inter + wo_b ----------
    # woT [F, D] column-block d fetched as ONE 3D DMA into [p, (kf j)] layout:
    # wo_big[p, kf*128 + j] = woT[kf*128 + p, d*128 + j]
    wo_src = io["woT"].rearrange("(kf p) d -> p kf d", p=P)
    for d in range(KD):
        wo_big = wop.tile([P, F], BF16, tag="wo", name=f"wo{d}")
        nc.sync.dma_start(out=wo_big.rearrange("p (kf j) -> p kf j", kf=KF),
                          in_=wo_src[:, :, d * P:(d + 1) * P])
        for th in range(TH):
            ts = slice(th * 512, (th + 1) * 512)
            psum2 = dps.tile([P, 512], F32, tag="p2")
            for kf in range(KF):
                nc.tensor.matmul(psum2, lhsT=wo_big[:, kf * P:(kf + 1) * P],
                                 rhs=inter_sb[kf][:, ts],
                                 start=(kf == 0), stop=(kf == KF - 1))
            osb = outp.tile([P, 512], F32, tag="osb")
            nc.vector.tensor_scalar(osb, psum2, wob_sb[:, d:d + 1], None,
                                    mybir.AluOpType.add)
            nc.gpsimd.dma_start(out=io["outT"][d * P:(d + 1) * P, ts], in_=osb)


_CACHED_NC = None


def build_nc():
    global _CACHED_NC
    if _CACHED_NC is not None:
        return _CACHED_NC
    nc = bacc.Bacc("TRN2", target_bir_lowering=False, debug=False,
                   enable_asserts=False, num_devices=NCORES)
    decls = [
        ("xT32", [D, N], F32, False),
        ("xT16", [D, N], BF16, False),
        ("gT", [D, E], F32, False),
        ("gb", [E], F32, False),
        ("aT", [D, ER], BF16, False),
        ("bT", [ER, F], BF16, False),
        ("wiT", [D, F], BF16, False),
        ("wib", [F], F32, False),
        ("woT", [F, D], BF16, False),
        ("wob", [D], F32, False),
        ("outT", [D, N], F32, True),
    ]
    io = {}
    for name, shape, dt_, is_out in decls:
        io[name] = nc.dram_tensor(
            name, shape, dt_, kind="ExternalOutput" if is_out else "ExternalInput"
        ).ap()
    with tile.TileContext(nc) as tc:
        with ExitStack() as ctx:
            _emit(ctx, tc, io)
    nc.compile()
    _CACHED_NC = nc
    return nc


def make_in_maps(inputs: dict) -> list[dict]:
    f32 = np.float32
    x = np.ascontiguousarray(np.asarray(inputs["hidden_states"], f32).reshape(NT, D))
    gT = np.ascontiguousarray(np.asarray(inputs["gate_W"], f32).T)          # [D, E]
    gb = np.ascontiguousarray(np.asarray(inputs["gate_b"], f32))            # [E]
    aT = np.ascontiguousarray(
        np.asarray(inputs["lora_A"], f32).reshape(ER, D).T.astype(BF))      # [D, 32]
    bT = np.ascontiguousarray(
        np.asarray(inputs["lora_B"], f32).transpose(0, 2, 1).reshape(ER, F).astype(BF))
    wiT = np.ascontiguousarray(np.asarray(inputs["wi_W"], f32).T.astype(BF))  # [D, F]
    wib = np.ascontiguousarray(np.asarray(inputs["wi_b"], f32))             # [F]
    woT = np.ascontiguousarray(np.asarray(inputs["wo_W"], f32).T.astype(BF))  # [F, D]
    wob = np.ascontiguousarray(np.asarray(inputs["wo_b"], f32))             # [D]

    in_maps = []
    for c in range(NCORES):
        xT32 = np.ascontiguousarray(x[c * N:(c + 1) * N].T)                 # [D, N]
        in_maps.append({
            "xT32": xT32,
            "xT16": np.ascontiguousarray(xT32.astype(BF)),
            "gT": gT, "gb": gb, "aT": aT, "bT": bT,
            "wiT": wiT, "wib": wib, "woT": woT, "wob": wob,
        })
    return in_maps


def kernel(**inputs) -> np.ndarray:
    nc = build_nc()
    in_maps = make_in_maps(inputs)
    res = run_bass_kernel_spmd(nc, in_maps, core_ids=list(range(NCORES)))
    out = np.empty((NT, D), np.float32)
    for c in range(NCORES):
        out[c * N:(c + 1) * N] = res.results[c]["outT"].T
    return out.reshape(B, S, D)


# revision 9
# speedup vs baseline: 1.3019x; 1.2259x over previous
"""MoE block (top-1 routing, shared FFN + per-expert LoRA) on 8 TRN2 NeuronCores.

Strategy: data-parallel over the 8192 tokens (1024 tokens/core), weights
replicated. The reference's dense-then-mask expert loop collapses to:

    logits = x @ gate_W.T + gate_b ; e* = argmax(logits)        (fp32)
    u      = x @ A_cat.T                 [N, 32]                (bf16)
    u_m    = u * onehot-mask(e*)  (zero all but selected expert's 4 lora rows)
    inter  = relu(x @ wi_W.T + u_m @ B_cat + wi_b)              (bf16 matmul)
    out    = inter @ wo_W.T + wo_b                              (bf16 matmul)

Everything runs in transposed (feature-major) layout on chip; the host
pre-transposes the shards/weights and re-transposes the output.
"""

import numpy as np
import ml_dtypes
from contextlib import ExitStack

import concourse.bass as bass
import concourse.tile as tile
from concourse import bacc, mybir
from concourse.bass_utils import run_bass_kernel_spmd
from concourse.masks import make_identity

F32 = mybir.dt.float32
F32R = mybir.dt.float32r
BF16 = mybir.dt.bfloat16
U32 = mybir.dt.uint32
BF = ml_dtypes.bfloat16

B, S, D, F, E, R = 4, 2048, 1024, 4096, 8, 4
NCORES = 8
NT = B * S          # 8192 tokens total
N = NT // NCORES    # 1024 tokens per core
ER = E * R          # 32 lora rows
KD = D // 128       # 8 contraction tiles over D
KF = F // 128       # 32 contraction tiles over F
TT = N // 128       # 8 token tiles (routing)
TH = N // 512       # 2 token halves (matmul moving dim)
P = 128

Relu = mybir.ActivationFunctionType.Relu


def _emit(ctx: ExitStack, tc: tile.TileContext, io: dict):
    nc = tc.nc

    consts = ctx.enter_context(tc.tile_pool(name="consts", bufs=1))
    xpool = ctx.enter_context(tc.tile_pool(name="xpool", bufs=1))
    wipool = ctx.enter_context(tc.tile_pool(name="wipool", bufs=1))
    ipool = ctx.enter_context(tc.tile_pool(name="ipool", bufs=1))
    x32p = ctx.enter_context(tc.tile_pool(name="x32p", bufs=3))
    wop = ctx.enter_context(tc.tile_pool(name="wop", bufs=2))
    rwork = ctx.enter_context(tc.tile_pool(name="rwork", bufs=2))
    outp = ctx.enter_context(tc.tile_pool(name="outp", bufs=3))
    sps = ctx.enter_context(tc.tile_pool(name="sps", bufs=1, space="PSUM"))
    bps = ctx.enter_context(tc.tile_pool(name="bps", bufs=5, space="PSUM"))

    # ---------- constants ----------
    identity = consts.tile([P, P], F32, tag="identity")
    make_identity(nc, identity)
    # econst[p, e*R + r] = e  (expert id per lora row, replicated on free axis)
    econst = consts.tile([P, ER], F32, tag="econst")
    for e in range(E):
        nc.vector.memset(econst[:, e * R:(e + 1) * R], float(e))
    gateb_sb = consts.tile([P, E], F32, tag="gb")
    nc.sync.dma_start(out=gateb_sb, in_=io["gb"].partition_broadcast(P))
    wib_sb = consts.tile([P, KF], F32, tag="wib")  # col f = wi_b[f*128 + p]
    nc.sync.dma_start(out=wib_sb, in_=io["wib"].rearrange("(a p) -> p a", p=P))
    wob_sb = consts.tile([P, KD], F32, tag="wob")
    nc.sync.dma_start(out=wob_sb, in_=io["wob"].rearrange("(a p) -> p a", p=P))

    gate_sb = []
    acat_sb = []
    for k in range(KD):
        g = consts.tile([P, E], F32, tag=f"gt{k}")
        nc.sync.dma_start(out=g, in_=io["gT"][k * P:(k + 1) * P, :])
        gate_sb.append(g)
        a = consts.tile([P, ER], BF16, tag=f"ac{k}")
        nc.sync.dma_start(out=a, in_=io["aT"][k * P:(k + 1) * P, :])
        acat_sb.append(a)
    bcat_sb = consts.tile([ER, F], BF16, tag="bc")
    nc.sync.dma_start(out=bcat_sb, in_=io["bT"])

    # ---------- PE warm-up: release the HAM clock gate while DMAs land ----------
    warm_src = consts.tile([P, 512], BF16, tag="warm")
    nc.vector.memset(warm_src, 1.0)
    for w in range(12):
        psum_w = bps.tile([P, 512], F32, tag="pbig", name=f"pw{w}")
        nc.tensor.matmul(psum_w, lhsT=warm_src[:, 0:P], rhs=warm_src,
                         start=True, stop=True)

    # ---------- resident activations / weights ----------
    inter_sb = [ipool.tile([P, N], BF16, tag=f"inter{f}", name=f"inter{f}")
                for f in range(KF)]
    maskT4 = consts.tile([ER, N], F32, tag="maskT4")
    um16 = [consts.tile([ER, 512], BF16, tag=f"um{th}", name=f"um{th}")
            for th in range(TH)]

    # ---------- DMA priority order on the sync queue:
    #   x32 (router-critical) -> x16 -> wi; wo later; outs on gpsimd queue.
    x32t = []
    for k in range(KD):
        x32 = x32p.tile([P, N], F32, tag="x32", name=f"x32_{k}")
        nc.sync.dma_start(out=x32, in_=io["xT32"][k * P:(k + 1) * P, :])
        x32t.append(x32)
    x16 = []
    for k in range(KD):
        t = xpool.tile([P, N], BF16, tag=f"x16_{k}")
        nc.sync.dma_start(out=t, in_=io["xT16"][k * P:(k + 1) * P, :])
        x16.append(t)
    wi_sb = []
    for k in range(KD):
        t = wipool.tile([P, F], BF16, tag=f"wi{k}")
        nc.sync.dma_start(out=t, in_=io["wiT"][k * P:(k + 1) * P, :])
        wi_sb.append(t)

    # ---------- router (fp32 via f32r matmuls, feature-major then transpose) ----
    psum_lt = [sps.tile([E, 512], F32, tag=f"plt{th}", name=f"plt{th}")
               for th in range(TH)]
    for k in range(KD):
        for th in range(TH):
            ts = slice(th * 512, (th + 1) * 512)
            nc.tensor.matmul(psum_lt[th], lhsT=gate_sb[k], rhs=x32t[k][:, ts],
                             start=(k == 0), stop=(k == KD - 1))
    for th in range(TH):
        logitsT = rwork.tile([E, 512], F32, tag="lgT")
        nc.vector.tensor_copy(logitsT, psum_lt[th])
        for q in range(4):
            tt = th * 4 + q
            # transpose [8, 128] logit chunk to token-major [128, 8]
            psum_tr = sps.tile([P, E], F32, tag="pmask", name=f"ptr{tt}")
            nc.tensor.matmul(psum_tr, lhsT=logitsT[:, q * P:(q + 1) * P],
                             rhs=identity[0:E, 0:E], is_transpose=True,
                             start=True, stop=True)
            logits = rwork.tile([P, E], F32, tag="lg")
            nc.vector.tensor_add(logits, psum_tr, gateb_sb)
            max8 = rwork.tile([P, E], F32, tag="mx")
            nc.vector.max(out=max8, in_=logits)
            idx8 = rwork.tile([P, E], U32, tag="ix")
            nc.vector.max_index(idx8, max8, logits)
            idxf = rwork.tile([P, 1], F32, tag="if")
            nc.vector.tensor_copy(idxf, idx8[:, 0:1])
            # mask_rep[t, e*R+r] = (argmax == e)
            mask_rep = rwork.tile([P, ER], F32, tag="mr")
            nc.vector.tensor_scalar(mask_rep, econst, idxf, None,
                                    mybir.AluOpType.is_equal)
            psum_m = sps.tile([ER, P], F32, tag="pmask", name=f"pm{tt}")
            nc.tensor.transpose(psum_m, mask_rep, identity)
            nc.vector.tensor_copy(maskT4[:, tt * P:(tt + 1) * P], psum_m)

    # ---------- masked lora projection u (bf16) ----------
    TS = [slice(th * 512, (th + 1) * 512) for th in range(TH)]
    psum_u = [bps.tile([ER, 512], F32, tag="pbig", name=f"pu{th}")
              for th in range(TH)]
    for k in range(KD):
        for th in range(TH):
            nc.tensor.matmul(psum_u[th], lhsT=acat_sb[k], rhs=x16[k][:, TS[th]],
                             start=(k == 0), stop=(k == KD - 1))
    for th in range(TH):
        nc.vector.tensor_mul(um16[th], psum_u[th], maskT4[:, TS[th]])

    # ---------- matmul 1: interT = relu(wi @ x.T + Bcat.T @ u_m + wi_b) ------
    # th0/th1 paired per stationary tile so each weight load feeds 2 matmuls
    for f in range(KF):
        ps = [bps.tile([P, 512], F32, tag="pbig", name=f"p1_{f}_{th}")
              for th in range(TH)]
        for k in range(KD):
            for th in range(TH):
                nc.tensor.matmul(ps[th], lhsT=wi_sb[k][:, f * P:(f + 1) * P],
                                 rhs=x16[k][:, TS[th]],
                                 start=(k == 0), stop=False)
        for th in range(TH):
            nc.tensor.matmul(ps[th], lhsT=bcat_sb[:, f * P:(f + 1) * P],
                             rhs=um16[th], start=False, stop=True)
        for th in range(TH):
            nc.scalar.activation(inter_sb[f][:, TS[th]], ps[th], Relu,
                                 bias=wib_sb[:, f:f + 1])

    # ---------- matmul 2: outT = wo @ inter + wo_b ----------
    # woT [F, D] column-block d fetched as ONE 3D DMA into [p, (kf j)] layout:
    # wo_big[p, kf*128 + j] = woT[kf*128 + p, d*128 + j]
    wo_src = io["woT"].rearrange("(kf p) d -> p kf d", p=P)
    for d in range(KD):
        wo_big = wop.tile([P, F], BF16, tag="wo", name=f"wo{d}")
        nc.sync.dma_start(out=wo_big.rearrange("p (kf j) -> p kf j", kf=KF),
                          in_=wo_src[:, :, d * P:(d + 1) * P])
        ps = [bps.tile([P, 512], F32, tag="pbig", name=f"p2_{d}_{th}")
              for th in range(TH)]
        for kf in range(KF):
            for th in range(TH):
                nc.tensor.matmul(ps[th], lhsT=wo_big[:, kf * P:(kf + 1) * P],
                                 rhs=inter_sb[kf][:, TS[th]],
                                 start=(kf == 0), stop=(kf == KF - 1))
        for th in range(TH):
            osb = outp.tile([P, 512], F32, tag="osb")
            nc.vector.tensor_scalar(osb, ps[th], wob_sb[:, d:d + 1], None,
                                    mybir.AluOpType.add)
            nc.gpsimd.dma_start(out=io["outT"][d * P:(d + 1) * P, TS[th]], in_=osb)


_CACHED_NC = None


def build_nc():
    global _CACHED_NC
    if _CACHED_NC is not None:
        return _CACHED_NC
    nc = bacc.Bacc("TRN2", target_bir_lowering=False, debug=False,
                   enable_asserts=False, num_devices=NCORES)
    decls = [
        ("xT32", [D, N], F32, False),
        ("xT16", [D, N], BF16, False),
        ("gT", [D, E], F32, False),
        ("gb", [E], F32, False),
        ("aT", [D, ER], BF16, False),
        ("bT", [ER, F], BF16, False),
        ("wiT", [D, F], BF16, False),
        ("wib", [F], F32, False),
        ("woT", [F, D], BF16, False),
        ("wob", [D], F32, False),
        ("outT", [D, N], F32, True),
    ]
    io = {}
    for name, shape, dt_, is_out in decls:
        io[name] = nc.dram_tensor(
            name, shape, dt_, kind="ExternalOutput" if is_out else "ExternalInput"
        ).ap()
    with tile.TileContext(nc) as tc:
        with ExitStack() as ctx:
            _emit(ctx, tc, io)
    nc.compile()
    _CACHED_NC = nc
    return nc


def make_in_maps(inputs: dict) -> list[dict]:
    f32 = np.float32
    x = np.ascontiguousarray(np.asarray(inputs["hidden_states"], f32).reshape(NT, D))
    gT = np.ascontiguousarray(np.asarray(inputs["gate_W"], f32).T)          # [D, E]
    gb = np.ascontiguousarray(np.asarray(inputs["gate_b"], f32))            # [E]
    aT = np.ascontiguousarray(
        np.asarray(inputs["lora_A"], f32).reshape(ER, D).T.astype(BF))      # [D, 32]
    bT = np.ascontiguousarray(
        np.asarray(inputs["lora_B"], f32).transpose(0, 2, 1).reshape(ER, F).astype(BF))
    wiT = np.ascontiguousarray(np.asarray(inputs["wi_W"], f32).T.astype(BF))  # [D, F]
    wib = np.ascontiguousarray(np.asarray(inputs["wi_b"], f32))             # [F]
    woT = np.ascontiguousarray(np.asarray(inputs["wo_W"], f32).T.astype(BF))  # [F, D]
    wob = np.ascontiguousarray(np.asarray(inputs["wo_b"], f32))             # [D]

    in_maps = []
    for c in range(NCORES):
        xT32 = np.ascontiguousarray(x[c * N:(c + 1) * N].T)                 # [D, N]
        in_maps.append({
            "xT32": xT32,
            "xT16": np.ascontiguousarray(xT32.astype(BF)),
            "gT": gT, "gb": gb, "aT": aT, "bT": bT,
            "wiT": wiT, "wib": wib, "woT": woT, "wob": wob,
        })
    return in_maps


def kernel(**inputs) -> np.ndarray:
    nc = build_nc()
    in_maps = make_in_maps(inputs)
    res = run_bass_kernel_spmd(nc, in_maps, core_ids=list(range(NCORES)))
    out = np.empty((NT, D), np.float32)
    for c in range(NCORES):
        out[c * N:(c + 1) * N] = res.results[c]["outT"].T
    return out.reshape(B, S, D)
